# revision 19
# baseline (speedup 1.0000x reference)
"""Trainium2 Bass kernel for batched dot-product attention.

reference semantics (fp32):
    scores = (Q @ K^T) / 128 + mask * -1e9      # mask in {0,1}, 1 = masked
    attn   = softmax(scores, axis=-1)
    out    = attn @ V
    returns (out, attn)

Sharding: B*H = 64 (batch, head) blocks split 8-per-core across 8 NeuronCores.
Each core's 8 heads share one batch's mask ([2048, 2048]).

Per-core kernel layout decisions:
  - scores kept in natural [q, k] layout (q on partitions) so the softmax
    reduction runs along the free axis and the 16 MB attn output DMAs out
    with contiguous 8 KB rows.
  - QK^T via PE with both operands bitcast to float32r (1 cycle/row at
    free-dim 512 instead of fp32's 4).
  - the additive mask ( -1e9 * m, pre-scaled by the 1/128 that the exp
    activation folds in ) is accumulated into the scores PSUM by a second
    matmul: lhsT = (-128e9 * I) [bf16], rhs = mask tile. Exact because
    m is 0/1 and exp underflows to +0 either way.
  - exp on ScalarE with scale=1/128 and accum_out producing the row-sum for
    free; normalize in place on VectorE; the normalized tile is both DMA'd
    out as `attn` and PE-transposed to feed the attn @ V matmul
    (contraction must sit on partitions).
  - attn @ V computed transposed (out^T[d, q] accum over k-tiles, V tile
    stationary, attn^T moving at N=512) then PE-transposed back.
"""

import numpy as np
from contextlib import ExitStack

try:
    import concourse.bass as bass
except ImportError:  # pragma: no cover
    import sys

    sys.path.insert(0, "/opt/trn_rl_repo")
    import concourse.bass as bass

import concourse.tile as tile
from concourse import bacc, mybir
from concourse.masks import make_identity
from concourse.bass_utils import run_bass_kernel_spmd

B, H, S, D = 4, 16, 2048, 128
NCORES = 8
HPC = (B * H) // NCORES  # heads per core = 8

F32 = mybir.dt.float32
F32R = mybir.dt.float32r
BF16 = mybir.dt.bfloat16

NEG = -128.0e9  # becomes -1e9 after the exp's 1/128 input scale


def build_attention_bass(hpc=HPC, s=S, d=D, hgrp=4, qchunk=512):
    """Build the per-core Bass program. All cores run the same program on
    different data (pure SPMD, no collectives)."""
    assert d == 128 and s % 512 == 0 and qchunk % 128 == 0 and s % qchunk == 0
    assert hpc % hgrp == 0
    nkt = s // 128  # k-tiles (contraction tiles for attn@V)
    nnb = s // 512  # 512-wide blocks of the score row
    nch = s // qchunk  # q chunks
    nqt = qchunk // 128  # q-tiles per chunk

    nc = bacc.Bacc(
        "TRN2", target_bir_lowering=False, debug=False, num_devices=NCORES
    )
    q_d = nc.dram_tensor("q", [hpc, s, d], F32, kind="ExternalInput").ap()
    k_d = nc.dram_tensor("k", [hpc, s, d], F32, kind="ExternalInput").ap()
    v_d = nc.dram_tensor("v", [hpc, s, d], F32, kind="ExternalInput").ap()
    m_d = nc.dram_tensor("mask", [s, s], F32, kind="ExternalInput").ap()
    out_d = nc.dram_tensor("out", [hpc, s, d], F32, kind="ExternalOutput").ap()
    attn_d = nc.dram_tensor("attn", [hpc, s, s], F32, kind="ExternalOutput").ap()

    with tile.TileContext(nc) as tc, ExitStack() as ctx:
        _emit(ctx, tc, nc, q_d, k_d, v_d, m_d, out_d, attn_d,
              hpc, s, d, hgrp, qchunk, nkt, nnb, nch, nqt)
    nc.compile()
    return nc


def _emit(ctx, tc, nc, q_d, k_d, v_d, m_d, out_d, attn_d,
          hpc, s, d, hgrp, qchunk, nkt, nnb, nch, nqt):
    EXP = mybir.ActivationFunctionType.Exp

    consts = ctx.enter_context(tc.tile_pool(name="consts", bufs=1))
    ident = consts.tile([128, 128], F32)
    make_identity(nc, ident[:, :])
    neg_ident = consts.tile([128, 128], BF16)
    nc.scalar.mul(neg_ident[:, :], ident[:, :], NEG)
    ident_r = consts.tile([128, 128], F32R)
    nc.vector.tensor_copy(ident_r[:, :], ident[:, :])

    kv_pool = ctx.enter_context(tc.tile_pool(name="kv", bufs=1))
    knat_pool = ctx.enter_context(tc.tile_pool(name="knat", bufs=2))
    mask_pool = ctx.enter_context(tc.tile_pool(name="maskp", bufs=2))
    p_pool = ctx.enter_context(tc.tile_pool(name="pp", bufs=nqt + 1))
    attnw = ctx.enter_context(tc.tile_pool(name="attnw", bufs=2))
    pt_pool = ctx.enter_context(tc.tile_pool(name="ptp", bufs=1))
    qt_pool = ctx.enter_context(tc.tile_pool(name="qtp", bufs=2))
    small = ctx.enter_context(tc.tile_pool(name="small", bufs=4))
    outw = ctx.enter_context(tc.tile_pool(name="outw", bufs=1))

    ps_s = ctx.enter_context(tc.tile_pool(name="ps_s", bufs=2, space="PSUM"))
    ps_pt = ctx.enter_context(tc.tile_pool(name="ps_pt", bufs=2, space="PSUM"))
    ps_av = ctx.enter_context(tc.tile_pool(name="ps_av", bufs=1, space="PSUM"))
    ps_sm = ctx.enter_context(tc.tile_pool(name="ps_sm", bufs=1, space="PSUM"))

    for g in range(hpc // hgrp):
        heads = list(range(g * hgrp, (g + 1) * hgrp))

        # residency for this head group: K^T ([d, k] layout) and V (natural)
        kT_sb = kv_pool.tile([128, hgrp, s], F32R, tag="kT")
        v_sb = kv_pool.tile([128, hgrp, nkt, 128], F32R, tag="v")
        for hi, h in enumerate(heads):
            nc.gpsimd.dma_start(
                out=v_sb[:, hi],
                in_=v_d[h].rearrange("(t p) d -> p t d", p=128),
            )
            k_nat = knat_pool.tile([128, nkt, 128], F32, tag="knat")
            nc.sync.dma_start(
                out=k_nat,
                in_=k_d[h].rearrange("(t p) d -> p t d", p=128),
            )
            for t in range(nkt):
                pst = ps_sm.tile([128, 128], F32, tag="tp128")
                nc.tensor.transpose(pst[:, :], k_nat[:, t], ident[:, :])
                nc.vector.tensor_copy(kT_sb[:, hi, t * 128:(t + 1) * 128], pst[:, :])

        for ci in range(nch):
            # mask rows for this q chunk, cast to bf16 (exact for 0/1),
            # shared by all heads in the group. Loads are emitted one chunk
            # ahead: the SWDGE issue queue is paced by compute, so emitting
            # the load at its use site starts the ~25us transfer too late
            # and stalls the PE at every chunk boundary.
            if ci == 0:
                mask_sb = mask_pool.tile([128, nqt, s], BF16, tag="mask")
                nc.gpsimd.dma_start(
                    out=mask_sb,
                    in_=m_d[0:qchunk, :].rearrange("(t p) k -> p t k", p=128),
                )
            else:
                mask_sb = mask_next
            if ci + 1 < nch:
                mask_next = mask_pool.tile([128, nqt, s], BF16, tag="mask")
                nc.gpsimd.dma_start(
                    out=mask_next,
                    in_=m_d[(ci + 1) * qchunk:(ci + 2) * qchunk, :].rearrange(
                        "(t p) k -> p t k", p=128
                    ),
                )
            for hi, h in enumerate(heads):
                p_tiles = []
                recips = []
                for qt in range(nqt):
                    qrow = ci * qchunk + qt * 128
                    q_nat = qt_pool.tile([128, 128], F32, tag="qnat")
                    nc.gpsimd.dma_start(out=q_nat, in_=q_d[h, qrow:qrow + 128, :])
                    pqt = ps_sm.tile([128, 128], F32, tag="tp128")
                    nc.tensor.transpose(pqt[:, :], q_nat[:, :], ident[:, :])
                    qT = qt_pool.tile([128, 128], F32R, tag="qT")
                    nc.vector.tensor_copy(qT[:, :], pqt[:, :])

                    p_t = p_pool.tile([128, s], F32R, tag="p")
                    expblk = min(1024, s)
                    nhalf = s // expblk
                    rs_p = small.tile([128, nhalf], F32, tag="rsp")
                    for hf in range(nhalf):
                        s_ps = ps_s.tile([128, expblk], F32, tag="s")
                        for j in range(expblk // 512):
                            nb = hf * (expblk // 512) + j
                            nc.tensor.matmul(
                                s_ps[:, j * 512:(j + 1) * 512],
                                qT[:, :],
                                kT_sb[:, hi, nb * 512:(nb + 1) * 512],
                                start=True,
                                stop=False,
                            )
                            nc.tensor.matmul(
                                s_ps[:, j * 512:(j + 1) * 512],
                                neg_ident[:, :],
                                mask_sb[:, qt, nb * 512:(nb + 1) * 512],
                                start=False,
                                stop=True,
                            )
                        nc.scalar.activation(
                            p_t[:, hf * expblk:(hf + 1) * expblk],
                            s_ps[:, :],
                            EXP,
                            scale=1.0 / 128.0,
                            accum_out=rs_p[:, hf:hf + 1],
                        )
                    rowsum = small.tile([128, 1], F32, tag="rowsum")
                    nc.vector.tensor_reduce(
                        rowsum[:, :], rs_p[:, :],
                        axis=mybir.AxisListType.X, op=mybir.AluOpType.add,
                    )
                    recip = small.tile([128, 1], F32, tag="recip", bufs=2 * nqt + 2)
                    nc.vector.reciprocal(recip[:, :], rowsum[:, :])
                    p_tiles.append(p_t)
                    recips.append(recip)

                # build attn^T tiles [k, q] for the attn @ V contraction
                pt_sb = pt_pool.tile([128, nkt, qchunk], F32R, tag="pt")
                for kt in range(nkt):
                    ptps = ps_pt.tile([128, qchunk], F32R, tag="ptps")
                    for qt in range(nqt):
                        nc.tensor.transpose(
                            ptps[:, qt * 128:(qt + 1) * 128],
                            p_tiles[qt][:, kt * 128:(kt + 1) * 128],
                            ident_r[:, :],
                        )
                    if kt % 2 == 0:
                        nc.vector.tensor_copy(pt_sb[:, kt], ptps[:, :])
                    else:
                        nc.scalar.copy(pt_sb[:, kt], ptps[:, :])

                # attn rows: normalize raw exp rows into a staging tile and
                # store; PE transposes above read the raw rows so the PE
                # pipeline does not wait on the rowsum/recip chain.
                for qt in range(nqt):
                    qrow = ci * qchunk + qt * 128
                    attn_sb = attnw.tile([128, s], F32R, tag="attn_sb")
                    nc.vector.tensor_scalar_mul(
                        attn_sb[:, :], p_tiles[qt][:, :], recips[qt][:, :]
                    )
                    nc.sync.dma_start(
                        out=attn_d.bitcast(F32R)[h, qrow:qrow + 128, :],
                        in_=attn_sb[:, :],
                    )

                # out^T[d, q] = sum_k V[k, d]^T-contracted with attn^T[k, q]
                av_ps = ps_av.tile([128, qchunk], F32, tag="av")
                for kt in range(nkt):
                    nc.tensor.matmul(
                        av_ps[:, :],
                        v_sb[:, hi, kt],
                        pt_sb[:, kt],
                        start=(kt == 0),
                        stop=(kt == nkt - 1),
                    )
                outT = outw.tile([128, qchunk], F32, tag="outT")
                nc.vector.tensor_copy(outT[:, :], av_ps[:, :])
                fx = ps_pt.tile([128, qchunk], F32, tag="ptps")
                fx = fx.rearrange("p (t d) -> p t d", t=nqt)
                for sub in range(nqt):
                    nc.tensor.transpose(
                        fx[:, sub], outT[:, sub * 128:(sub + 1) * 128], ident[:, :]
                    )
                o_sb = outw.tile([128, nqt, 128], F32, tag="osb")
                for sub in range(nqt):
                    nc.vector.tensor_scalar_mul(
                        o_sb[:, sub], fx[:, sub], recips[sub][:, :]
                    )
                nc.sync.dma_start(
                    out=out_d[h, ci * qchunk:(ci + 1) * qchunk, :].rearrange(
                        "(t p) d -> p t d", p=128
                    ),
                    in_=o_sb[:, :],
                )


_NC_CACHE = {}


def _get_nc():
    if "nc" not in _NC_CACHE:
        _NC_CACHE["nc"] = build_attention_bass()
    return _NC_CACHE["nc"]


def kernel(queries, keys, values, mask, _trace=False):
    queries = np.ascontiguousarray(np.asarray(queries, dtype=np.float32))
    keys = np.ascontiguousarray(np.asarray(keys, dtype=np.float32))
    values = np.ascontiguousarray(np.asarray(values, dtype=np.float32))
    mask = np.ascontiguousarray(np.asarray(mask, dtype=np.float32))

    qf = queries.reshape(B * H, S, D)
    kf = keys.reshape(B * H, S, D)
    vf = values.reshape(B * H, S, D)

    in_maps = []
    for c in range(NCORES):
        lo = c * HPC
        in_maps.append(
            {
                "q": qf[lo:lo + HPC],
                "k": kf[lo:lo + HPC],
                "v": vf[lo:lo + HPC],
                "mask": mask[lo // H, 0],
            }
        )

    nc = _get_nc()
    res = None
    last_err = None
    for attempt in range(3):
        try:
            res = run_bass_kernel_spmd(
                nc, in_maps, list(range(NCORES)), trace=_trace
            )
            break
        except Exception as e:  # transient NRT device faults: reset + retry
            last_err = e
            try:
                import jax

                jax.clear_caches()
                jax.extend.backend.clear_backends()
            except Exception:
                pass
            import time as _time

            _time.sleep(5)
    if res is None:
        raise last_err

    out = np.empty((B * H, S, D), dtype=np.float32)
    attn = np.empty((B * H, S, S), dtype=np.float32)
    for c in range(NCORES):
        lo = c * HPC
        out[lo:lo + HPC] = res.results[c]["out"]
        attn[lo:lo + HPC] = res.results[c]["attn"]

    out = out.reshape(B, H, S, D)
    attn = attn.reshape(B, H, S, S)
    if _trace:
        return (out, attn), res
    return out, attn


# revision 20
# speedup vs baseline: 1.0480x; 1.0480x over previous
"""Trainium2 Bass kernel for batched dot-product attention.

reference semantics (fp32):
    scores = (Q @ K^T) / 128 + mask * -1e9      # mask in {0,1}, 1 = masked
    attn   = softmax(scores, axis=-1)
    out    = attn @ V
    returns (out, attn)

Sharding: B*H = 64 (batch, head) blocks split 8-per-core across 8 NeuronCores.
Each core's 8 heads share one batch's mask ([2048, 2048]).

Per-core kernel layout decisions:
  - scores kept in natural [q, k] layout (q on partitions) so the softmax
    reduction runs along the free axis and the 16 MB attn output DMAs out
    with contiguous 8 KB rows.
  - QK^T via PE with both operands bitcast to float32r (1 cycle/row at
    free-dim 512 instead of fp32's 4).
  - the additive mask ( -1e9 * m, pre-scaled by the 1/128 that the exp
    activation folds in ) is accumulated into the scores PSUM by a second
    matmul: lhsT = (-128e9 * I) [bf16], rhs = mask tile. Exact because
    m is 0/1 and exp underflows to +0 either way.
  - exp on ScalarE with scale=1/128 and accum_out producing the row-sum for
    free; normalize in place on VectorE; the normalized tile is both DMA'd
    out as `attn` and PE-transposed to feed the attn @ V matmul
    (contraction must sit on partitions).
  - attn @ V computed transposed (out^T[d, q] accum over k-tiles, V tile
    stationary, attn^T moving at N=512) then PE-transposed back.
"""

import numpy as np
from contextlib import ExitStack

try:
    import concourse.bass as bass
except ImportError:  # pragma: no cover
    import sys

    sys.path.insert(0, "/opt/trn_rl_repo")
    import concourse.bass as bass

import concourse.tile as tile
from concourse import bacc, mybir
from concourse.masks import make_identity
from concourse.bass_utils import run_bass_kernel_spmd

B, H, S, D = 4, 16, 2048, 128
NCORES = 8
HPC = (B * H) // NCORES  # heads per core = 8

F32 = mybir.dt.float32
F32R = mybir.dt.float32r
BF16 = mybir.dt.bfloat16

NEG = -128.0e9  # becomes -1e9 after the exp's 1/128 input scale


def build_attention_bass(hpc=HPC, s=S, d=D, hgrp=4, qchunk=512):
    """Build the per-core Bass program. All cores run the same program on
    different data (pure SPMD, no collectives)."""
    assert d == 128 and s % 512 == 0 and qchunk % 128 == 0 and s % qchunk == 0
    assert hpc % hgrp == 0
    nkt = s // 128  # k-tiles (contraction tiles for attn@V)
    nnb = s // 512  # 512-wide blocks of the score row
    nch = s // qchunk  # q chunks
    nqt = qchunk // 128  # q-tiles per chunk

    nc = bacc.Bacc(
        "TRN2", target_bir_lowering=False, debug=False, num_devices=NCORES
    )
    q_d = nc.dram_tensor("q", [hpc, s, d], F32, kind="ExternalInput").ap()
    k_d = nc.dram_tensor("k", [hpc, s, d], F32, kind="ExternalInput").ap()
    v_d = nc.dram_tensor("v", [hpc, s, d], F32, kind="ExternalInput").ap()
    m_d = nc.dram_tensor("mask", [s, s], F32, kind="ExternalInput").ap()
    out_d = nc.dram_tensor("out", [hpc, s, d], F32, kind="ExternalOutput").ap()
    attn_d = nc.dram_tensor("attn", [hpc, s, s], F32, kind="ExternalOutput").ap()

    with tile.TileContext(nc) as tc, ExitStack() as ctx:
        _emit(ctx, tc, nc, q_d, k_d, v_d, m_d, out_d, attn_d,
              hpc, s, d, hgrp, qchunk, nkt, nnb, nch, nqt)
    nc.compile()
    return nc


def _emit(ctx, tc, nc, q_d, k_d, v_d, m_d, out_d, attn_d,
          hpc, s, d, hgrp, qchunk, nkt, nnb, nch, nqt):
    EXP = mybir.ActivationFunctionType.Exp

    consts = ctx.enter_context(tc.tile_pool(name="consts", bufs=1))
    ident = consts.tile([128, 128], F32)
    make_identity(nc, ident[:, :])
    neg_ident = consts.tile([128, 128], BF16)
    nc.scalar.mul(neg_ident[:, :], ident[:, :], NEG)
    ident_r = consts.tile([128, 128], F32R)
    nc.vector.tensor_copy(ident_r[:, :], ident[:, :])

    kv_pool = ctx.enter_context(tc.tile_pool(name="kv", bufs=1))
    knat_pool = ctx.enter_context(tc.tile_pool(name="knat", bufs=2))
    mask_pool = ctx.enter_context(tc.tile_pool(name="maskp", bufs=2))
    p_pool = ctx.enter_context(tc.tile_pool(name="pp", bufs=nqt + 1))
    attnw = ctx.enter_context(tc.tile_pool(name="attnw", bufs=2))
    pt_pool = ctx.enter_context(tc.tile_pool(name="ptp", bufs=1))
    qt_pool = ctx.enter_context(tc.tile_pool(name="qtp", bufs=4))
    small = ctx.enter_context(tc.tile_pool(name="small", bufs=4))
    outw = ctx.enter_context(tc.tile_pool(name="outw", bufs=1))

    ps_s = ctx.enter_context(tc.tile_pool(name="ps_s", bufs=2, space="PSUM"))
    ps_pt = ctx.enter_context(tc.tile_pool(name="ps_pt", bufs=2, space="PSUM"))
    ps_av = ctx.enter_context(tc.tile_pool(name="ps_av", bufs=1, space="PSUM"))
    ps_sm = ctx.enter_context(tc.tile_pool(name="ps_sm", bufs=1, space="PSUM"))

    for g in range(hpc // hgrp):
        heads = list(range(g * hgrp, (g + 1) * hgrp))

        # residency for this head group: K^T ([d, k] layout) and V (natural)
        kT_sb = kv_pool.tile([128, hgrp, s], F32R, tag="kT")
        v_sb = kv_pool.tile([128, hgrp, nkt, 128], F32R, tag="v")
        for hi, h in enumerate(heads):
            nc.gpsimd.dma_start(
                out=v_sb[:, hi],
                in_=v_d[h].rearrange("(t p) d -> p t d", p=128),
            )
            nh = nkt // 2
            for half in range(2):
                k_nat = knat_pool.tile([128, nh, 128], F32, tag="knat")
                r0 = half * nh * 128
                nc.sync.dma_start(
                    out=k_nat,
                    in_=k_d[h, r0:r0 + nh * 128, :].rearrange(
                        "(t p) d -> p t d", p=128
                    ),
                )
                for t in range(nh):
                    tg = half * nh + t
                    pst = ps_sm.tile([128, 128], F32, tag="tp128")
                    nc.tensor.transpose(pst[:, :], k_nat[:, t], ident[:, :])
                    nc.vector.tensor_copy(
                        kT_sb[:, hi, tg * 128:(tg + 1) * 128], pst[:, :]
                    )

        for ci in range(nch):
            # mask rows for this q chunk, cast to bf16 (exact for 0/1),
            # shared by all heads in the group. Loads are emitted one chunk
            # ahead: the SWDGE issue queue is paced by compute, so emitting
            # the load at its use site starts the ~25us transfer too late
            # and stalls the PE at every chunk boundary.
            if ci == 0:
                mask_sb = mask_pool.tile([128, nqt, s], BF16, tag="mask")
                nc.gpsimd.dma_start(
                    out=mask_sb,
                    in_=m_d[0:qchunk, :].rearrange("(t p) k -> p t k", p=128),
                )
            else:
                mask_sb = mask_next
            if ci + 1 < nch:
                mask_next = mask_pool.tile([128, nqt, s], BF16, tag="mask")
                nc.gpsimd.dma_start(
                    out=mask_next,
                    in_=m_d[(ci + 1) * qchunk:(ci + 2) * qchunk, :].rearrange(
                        "(t p) k -> p t k", p=128
                    ),
                )
            for hi, h in enumerate(heads):
                p_tiles = []
                recips = []
                for qt in range(nqt):
                    qrow = ci * qchunk + qt * 128
                    q_nat = qt_pool.tile([128, 128], F32, tag="qnat")
                    nc.gpsimd.dma_start(out=q_nat, in_=q_d[h, qrow:qrow + 128, :])
                    pqt = ps_sm.tile([128, 128], F32, tag="tp128")
                    nc.tensor.transpose(pqt[:, :], q_nat[:, :], ident[:, :])
                    qT = qt_pool.tile([128, 128], F32R, tag="qT")
                    nc.vector.tensor_copy(qT[:, :], pqt[:, :])

                    p_t = p_pool.tile([128, s], F32R, tag="p")
                    expblk = min(1024, s)
                    nhalf = s // expblk
                    rs_p = small.tile([128, nhalf], F32, tag="rsp")
                    for hf in range(nhalf):
                        s_ps = ps_s.tile([128, expblk], F32, tag="s")
                        for j in range(expblk // 512):
                            nb = hf * (expblk // 512) + j
                            nc.tensor.matmul(
                                s_ps[:, j * 512:(j + 1) * 512],
                                qT[:, :],
                                kT_sb[:, hi, nb * 512:(nb + 1) * 512],
                                start=True,
                                stop=False,
                            )
                            nc.tensor.matmul(
                                s_ps[:, j * 512:(j + 1) * 512],
                                neg_ident[:, :],
                                mask_sb[:, qt, nb * 512:(nb + 1) * 512],
                                start=False,
                                stop=True,
                            )
                        nc.scalar.activation(
                            p_t[:, hf * expblk:(hf + 1) * expblk],
                            s_ps[:, :],
                            EXP,
                            scale=1.0 / 128.0,
                            accum_out=rs_p[:, hf:hf + 1],
                        )
                    rowsum = small.tile([128, 1], F32, tag="rowsum")
                    nc.vector.tensor_reduce(
                        rowsum[:, :], rs_p[:, :],
                        axis=mybir.AxisListType.X, op=mybir.AluOpType.add,
                    )
                    recip = small.tile([128, 1], F32, tag="recip", bufs=2 * nqt + 2)
                    nc.vector.reciprocal(recip[:, :], rowsum[:, :])
                    p_tiles.append(p_t)
                    recips.append(recip)

                # build attn^T tiles [k, q] for the attn @ V contraction
                pt_sb = pt_pool.tile([128, nkt, qchunk], F32R, tag="pt")
                for kt in range(nkt):
                    ptps = ps_pt.tile([128, qchunk], F32R, tag="ptps")
                    for qt in range(nqt):
                        nc.tensor.transpose(
                            ptps[:, qt * 128:(qt + 1) * 128],
                            p_tiles[qt][:, kt * 128:(kt + 1) * 128],
                            ident_r[:, :],
                        )
                    if kt % 2 == 0:
                        nc.vector.tensor_copy(pt_sb[:, kt], ptps[:, :])
                    else:
                        nc.scalar.copy(pt_sb[:, kt], ptps[:, :])

                # attn rows: normalize raw exp rows into a staging tile and
                # store; PE transposes above read the raw rows so the PE
                # pipeline does not wait on the rowsum/recip chain.
                for qt in range(nqt):
                    qrow = ci * qchunk + qt * 128
                    attn_sb = attnw.tile([128, s], F32R, tag="attn_sb")
                    nc.vector.tensor_scalar_mul(
                        attn_sb[:, :], p_tiles[qt][:, :], recips[qt][:, :]
                    )
                    nc.sync.dma_start(
                        out=attn_d.bitcast(F32R)[h, qrow:qrow + 128, :],
                        in_=attn_sb[:, :],
                    )

                # out^T[d, q] = sum_k V[k, d]^T-contracted with attn^T[k, q]
                av_ps = ps_av.tile([128, qchunk], F32, tag="av")
                for kt in range(nkt):
                    nc.tensor.matmul(
                        av_ps[:, :],
                        v_sb[:, hi, kt],
                        pt_sb[:, kt],
                        start=(kt == 0),
                        stop=(kt == nkt - 1),
                    )
                outT = outw.tile([128, qchunk], F32, tag="outT")
                nc.vector.tensor_copy(outT[:, :], av_ps[:, :])
                fx = ps_pt.tile([128, qchunk], F32, tag="ptps")
                fx = fx.rearrange("p (t d) -> p t d", t=nqt)
                for sub in range(nqt):
                    nc.tensor.transpose(
                        fx[:, sub], outT[:, sub * 128:(sub + 1) * 128], ident[:, :]
                    )
                o_sb = outw.tile([128, nqt, 128], F32, tag="osb")
                for sub in range(nqt):
                    nc.vector.tensor_scalar_mul(
                        o_sb[:, sub], fx[:, sub], recips[sub][:, :]
                    )
                nc.sync.dma_start(
                    out=out_d[h, ci * qchunk:(ci + 1) * qchunk, :].rearrange(
                        "(t p) d -> p t d", p=128
                    ),
                    in_=o_sb[:, :],
                )


_NC_CACHE = {}


def _get_nc():
    if "nc" not in _NC_CACHE:
        _NC_CACHE["nc"] = build_attention_bass()
    return _NC_CACHE["nc"]


def kernel(queries, keys, values, mask, _trace=False):
    queries = np.ascontiguousarray(np.asarray(queries, dtype=np.float32))
    keys = np.ascontiguousarray(np.asarray(keys, dtype=np.float32))
    values = np.ascontiguousarray(np.asarray(values, dtype=np.float32))
    mask = np.ascontiguousarray(np.asarray(mask, dtype=np.float32))

    qf = queries.reshape(B * H, S, D)
    kf = keys.reshape(B * H, S, D)
    vf = values.reshape(B * H, S, D)

    in_maps = []
    for c in range(NCORES):
        lo = c * HPC
        in_maps.append(
            {
                "q": qf[lo:lo + HPC],
                "k": kf[lo:lo + HPC],
                "v": vf[lo:lo + HPC],
                "mask": mask[lo // H, 0],
            }
        )

    nc = _get_nc()
    res = None
    last_err = None
    for attempt in range(3):
        try:
            res = run_bass_kernel_spmd(
                nc, in_maps, list(range(NCORES)), trace=_trace
            )
            break
        except Exception as e:  # transient NRT device faults: reset + retry
            last_err = e
            try:
                import jax

                jax.clear_caches()
                jax.extend.backend.clear_backends()
            except Exception:
                pass
            import time as _time

            _time.sleep(5)
    if res is None:
        raise last_err

    out = np.empty((B * H, S, D), dtype=np.float32)
    attn = np.empty((B * H, S, S), dtype=np.float32)
    for c in range(NCORES):
        lo = c * HPC
        out[lo:lo + HPC] = res.results[c]["out"]
        attn[lo:lo + HPC] = res.results[c]["attn"]

    out = out.reshape(B, H, S, D)
    attn = attn.reshape(B, H, S, S)
    if _trace:
        return (out, attn), res
    return out, attn


# revision 21
# speedup vs baseline: 1.0559x; 1.0076x over previous
"""Trainium2 Bass kernel for batched dot-product attention.

reference semantics (fp32):
    scores = (Q @ K^T) / 128 + mask * -1e9      # mask in {0,1}, 1 = masked
    attn   = softmax(scores, axis=-1)
    out    = attn @ V
    returns (out, attn)

Sharding: B*H = 64 (batch, head) blocks split 8-per-core across 8 NeuronCores.
Each core's 8 heads share one batch's mask ([2048, 2048]).

Per-core kernel layout decisions:
  - scores kept in natural [q, k] layout (q on partitions) so the softmax
    reduction runs along the free axis and the 16 MB attn output DMAs out
    with contiguous 8 KB rows.
  - QK^T via PE with both operands bitcast to float32r (1 cycle/row at
    free-dim 512 instead of fp32's 4).
  - the additive mask ( -1e9 * m, pre-scaled by the 1/128 that the exp
    activation folds in ) is accumulated into the scores PSUM by a second
    matmul: lhsT = (-128e9 * I) [bf16], rhs = mask tile. Exact because
    m is 0/1 and exp underflows to +0 either way.
  - exp on ScalarE with scale=1/128 and accum_out producing the row-sum for
    free; normalize in place on VectorE; the normalized tile is both DMA'd
    out as `attn` and PE-transposed to feed the attn @ V matmul
    (contraction must sit on partitions).
  - attn @ V computed transposed (out^T[d, q] accum over k-tiles, V tile
    stationary, attn^T moving at N=512) then PE-transposed back.
"""

import numpy as np
from contextlib import ExitStack

try:
    import concourse.bass as bass
except ImportError:  # pragma: no cover
    import sys

    sys.path.insert(0, "/opt/trn_rl_repo")
    import concourse.bass as bass

import concourse.tile as tile
from concourse import bacc, mybir
from concourse.masks import make_identity
from concourse.bass_utils import run_bass_kernel_spmd

B, H, S, D = 4, 16, 2048, 128
NCORES = 8
HPC = (B * H) // NCORES  # heads per core = 8

F32 = mybir.dt.float32
F32R = mybir.dt.float32r
BF16 = mybir.dt.bfloat16

NEG = -128.0e9  # becomes -1e9 after the exp's 1/128 input scale


def build_attention_bass(hpc=HPC, s=S, d=D, hgrp=4, qchunk=512):
    """Build the per-core Bass program. All cores run the same program on
    different data (pure SPMD, no collectives)."""
    assert d == 128 and s % 512 == 0 and qchunk % 128 == 0 and s % qchunk == 0
    assert hpc % hgrp == 0
    nkt = s // 128  # k-tiles (contraction tiles for attn@V)
    nnb = s // 512  # 512-wide blocks of the score row
    nch = s // qchunk  # q chunks
    nqt = qchunk // 128  # q-tiles per chunk

    nc = bacc.Bacc(
        "TRN2", target_bir_lowering=False, debug=False, num_devices=NCORES
    )
    q_d = nc.dram_tensor("q", [hpc, s, d], F32, kind="ExternalInput").ap()
    k_d = nc.dram_tensor("k", [hpc, s, d], F32, kind="ExternalInput").ap()
    v_d = nc.dram_tensor("v", [hpc, s, d], F32, kind="ExternalInput").ap()
    m_d = nc.dram_tensor("mask", [s, s], F32, kind="ExternalInput").ap()
    out_d = nc.dram_tensor("out", [hpc, s, d], F32, kind="ExternalOutput").ap()
    attn_d = nc.dram_tensor("attn", [hpc, s, s], F32, kind="ExternalOutput").ap()

    with tile.TileContext(nc) as tc, ExitStack() as ctx:
        _emit(ctx, tc, nc, q_d, k_d, v_d, m_d, out_d, attn_d,
              hpc, s, d, hgrp, qchunk, nkt, nnb, nch, nqt)
    nc.compile()
    return nc


def _emit(ctx, tc, nc, q_d, k_d, v_d, m_d, out_d, attn_d,
          hpc, s, d, hgrp, qchunk, nkt, nnb, nch, nqt):
    EXP = mybir.ActivationFunctionType.Exp

    consts = ctx.enter_context(tc.tile_pool(name="consts", bufs=1))
    ident = consts.tile([128, 128], F32)
    make_identity(nc, ident[:, :])
    neg_ident = consts.tile([128, 128], BF16)
    nc.scalar.mul(neg_ident[:, :], ident[:, :], NEG)
    ident_r = consts.tile([128, 128], F32R)
    nc.vector.tensor_copy(ident_r[:, :], ident[:, :])

    kv_pool = ctx.enter_context(tc.tile_pool(name="kv", bufs=1))
    knat_pool = ctx.enter_context(tc.tile_pool(name="knat", bufs=2))
    mask_pool = ctx.enter_context(tc.tile_pool(name="maskp", bufs=2))
    p_pool = ctx.enter_context(tc.tile_pool(name="pp", bufs=nqt + 1))
    attnw = ctx.enter_context(tc.tile_pool(name="attnw", bufs=2))
    pt_pool = ctx.enter_context(tc.tile_pool(name="ptp", bufs=1))
    qt_pool = ctx.enter_context(tc.tile_pool(name="qtp", bufs=4))
    small = ctx.enter_context(tc.tile_pool(name="small", bufs=4))
    outw = ctx.enter_context(tc.tile_pool(name="outw", bufs=2))

    ps_s = ctx.enter_context(tc.tile_pool(name="ps_s", bufs=2, space="PSUM"))
    ps_pt = ctx.enter_context(tc.tile_pool(name="ps_pt", bufs=2, space="PSUM"))
    ps_av = ctx.enter_context(tc.tile_pool(name="ps_av", bufs=1, space="PSUM"))
    ps_sm = ctx.enter_context(tc.tile_pool(name="ps_sm", bufs=1, space="PSUM"))

    for g in range(hpc // hgrp):
        heads = list(range(g * hgrp, (g + 1) * hgrp))

        # residency for this head group: K^T ([d, k] layout) and V (natural)
        kT_sb = kv_pool.tile([128, hgrp, s], F32R, tag="kT")
        v_sb = kv_pool.tile([128, hgrp, nkt, 128], F32R, tag="v")
        for hi, h in enumerate(heads):
            nc.gpsimd.dma_start(
                out=v_sb[:, hi],
                in_=v_d[h].rearrange("(t p) d -> p t d", p=128),
            )
            nh = nkt // 2
            for half in range(2):
                k_nat = knat_pool.tile([128, nh, 128], F32, tag="knat")
                r0 = half * nh * 128
                nc.sync.dma_start(
                    out=k_nat,
                    in_=k_d[h, r0:r0 + nh * 128, :].rearrange(
                        "(t p) d -> p t d", p=128
                    ),
                )
                for t in range(nh):
                    tg = half * nh + t
                    pst = ps_sm.tile([128, 128], F32, tag="tp128")
                    nc.tensor.transpose(pst[:, :], k_nat[:, t], ident[:, :])
                    nc.vector.tensor_copy(
                        kT_sb[:, hi, tg * 128:(tg + 1) * 128], pst[:, :]
                    )

        for ci in range(nch):
            # mask rows for this q chunk, cast to bf16 (exact for 0/1),
            # shared by all heads in the group. Loads are emitted one chunk
            # ahead: the SWDGE issue queue is paced by compute, so emitting
            # the load at its use site starts the ~25us transfer too late
            # and stalls the PE at every chunk boundary.
            if ci == 0:
                mask_sb = mask_pool.tile([128, nqt, s], BF16, tag="mask")
                nc.gpsimd.dma_start(
                    out=mask_sb,
                    in_=m_d[0:qchunk, :].rearrange("(t p) k -> p t k", p=128),
                )
            else:
                mask_sb = mask_next
            if ci + 1 < nch:
                mask_next = mask_pool.tile([128, nqt, s], BF16, tag="mask")
                nc.gpsimd.dma_start(
                    out=mask_next,
                    in_=m_d[(ci + 1) * qchunk:(ci + 2) * qchunk, :].rearrange(
                        "(t p) k -> p t k", p=128
                    ),
                )
            for hi, h in enumerate(heads):
                p_tiles = []
                recips = []
                for qt in range(nqt):
                    qrow = ci * qchunk + qt * 128
                    q_nat = qt_pool.tile([128, 128], F32, tag="qnat")
                    nc.gpsimd.dma_start(out=q_nat, in_=q_d[h, qrow:qrow + 128, :])
                    pqt = ps_sm.tile([128, 128], F32, tag="tp128")
                    nc.tensor.transpose(pqt[:, :], q_nat[:, :], ident[:, :])
                    qT = qt_pool.tile([128, 128], F32R, tag="qT")
                    nc.vector.tensor_copy(qT[:, :], pqt[:, :])

                    p_t = p_pool.tile([128, s], F32R, tag="p")
                    expblk = min(1024, s)
                    nhalf = s // expblk
                    rs_p = small.tile([128, nhalf], F32, tag="rsp")
                    for hf in range(nhalf):
                        s_ps = ps_s.tile([128, expblk], F32, tag="s")
                        for j in range(expblk // 512):
                            nb = hf * (expblk // 512) + j
                            nc.tensor.matmul(
                                s_ps[:, j * 512:(j + 1) * 512],
                                qT[:, :],
                                kT_sb[:, hi, nb * 512:(nb + 1) * 512],
                                start=True,
                                stop=False,
                            )
                            nc.tensor.matmul(
                                s_ps[:, j * 512:(j + 1) * 512],
                                neg_ident[:, :],
                                mask_sb[:, qt, nb * 512:(nb + 1) * 512],
                                start=False,
                                stop=True,
                            )
                        nc.scalar.activation(
                            p_t[:, hf * expblk:(hf + 1) * expblk],
                            s_ps[:, :],
                            EXP,
                            scale=1.0 / 128.0,
                            accum_out=rs_p[:, hf:hf + 1],
                        )
                    rowsum = small.tile([128, 1], F32, tag="rowsum")
                    nc.vector.tensor_reduce(
                        rowsum[:, :], rs_p[:, :],
                        axis=mybir.AxisListType.X, op=mybir.AluOpType.add,
                    )
                    recip = small.tile([128, 1], F32, tag="recip", bufs=2 * nqt + 2)
                    nc.vector.reciprocal(recip[:, :], rowsum[:, :])
                    p_tiles.append(p_t)
                    recips.append(recip)

                # build attn^T tiles [k, q] for the attn @ V contraction
                pt_sb = pt_pool.tile([128, nkt, qchunk], F32R, tag="pt")
                for kt in range(nkt):
                    ptps = ps_pt.tile([128, qchunk], F32R, tag="ptps")
                    for qt in range(nqt):
                        nc.tensor.transpose(
                            ptps[:, qt * 128:(qt + 1) * 128],
                            p_tiles[qt][:, kt * 128:(kt + 1) * 128],
                            ident_r[:, :],
                        )
                    if kt % 2 == 0:
                        nc.vector.tensor_copy(pt_sb[:, kt], ptps[:, :])
                    else:
                        nc.scalar.copy(pt_sb[:, kt], ptps[:, :])

                # attn rows: normalize raw exp rows into a staging tile and
                # store; PE transposes above read the raw rows so the PE
                # pipeline does not wait on the rowsum/recip chain.
                for qt in range(nqt):
                    qrow = ci * qchunk + qt * 128
                    attn_sb = attnw.tile([128, s], F32R, tag="attn_sb")
                    nc.vector.tensor_scalar_mul(
                        attn_sb[:, :], p_tiles[qt][:, :], recips[qt][:, :]
                    )
                    nc.sync.dma_start(
                        out=attn_d.bitcast(F32R)[h, qrow:qrow + 128, :],
                        in_=attn_sb[:, :],
                    )

                # out^T[d, q] = sum_k V[k, d]^T-contracted with attn^T[k, q]
                av_ps = ps_av.tile([128, qchunk], F32, tag="av")
                for kt in range(nkt):
                    nc.tensor.matmul(
                        av_ps[:, :],
                        v_sb[:, hi, kt],
                        pt_sb[:, kt],
                        start=(kt == 0),
                        stop=(kt == nkt - 1),
                    )
                outT = outw.tile([128, qchunk], F32, tag="outT")
                nc.vector.tensor_copy(outT[:, :], av_ps[:, :])
                fx = ps_pt.tile([128, qchunk], F32, tag="ptps")
                fx = fx.rearrange("p (t d) -> p t d", t=nqt)
                for sub in range(nqt):
                    nc.tensor.transpose(
                        fx[:, sub], outT[:, sub * 128:(sub + 1) * 128], ident[:, :]
                    )
                o_sb = outw.tile([128, nqt, 128], F32, tag="osb")
                for sub in range(nqt):
                    nc.vector.tensor_scalar_mul(
                        o_sb[:, sub], fx[:, sub], recips[sub][:, :]
                    )
                nc.sync.dma_start(
                    out=out_d[h, ci * qchunk:(ci + 1) * qchunk, :].rearrange(
                        "(t p) d -> p t d", p=128
                    ),
                    in_=o_sb[:, :],
                )


_NC_CACHE = {}


def _get_nc():
    if "nc" not in _NC_CACHE:
        _NC_CACHE["nc"] = build_attention_bass()
    return _NC_CACHE["nc"]


def kernel(queries, keys, values, mask, _trace=False):
    queries = np.ascontiguousarray(np.asarray(queries, dtype=np.float32))
    keys = np.ascontiguousarray(np.asarray(keys, dtype=np.float32))
    values = np.ascontiguousarray(np.asarray(values, dtype=np.float32))
    mask = np.ascontiguousarray(np.asarray(mask, dtype=np.float32))

    qf = queries.reshape(B * H, S, D)
    kf = keys.reshape(B * H, S, D)
    vf = values.reshape(B * H, S, D)

    in_maps = []
    for c in range(NCORES):
        lo = c * HPC
        in_maps.append(
            {
                "q": qf[lo:lo + HPC],
                "k": kf[lo:lo + HPC],
                "v": vf[lo:lo + HPC],
                "mask": mask[lo // H, 0],
            }
        )

    nc = _get_nc()
    res = None
    last_err = None
    for attempt in range(3):
        try:
            res = run_bass_kernel_spmd(
                nc, in_maps, list(range(NCORES)), trace=_trace
            )
            break
        except Exception as e:  # transient NRT device faults: reset + retry
            last_err = e
            try:
                import jax

                jax.clear_caches()
                jax.extend.backend.clear_backends()
            except Exception:
                pass
            import time as _time

            _time.sleep(5)
    if res is None:
        raise last_err

    out = np.empty((B * H, S, D), dtype=np.float32)
    attn = np.empty((B * H, S, S), dtype=np.float32)
    for c in range(NCORES):
        lo = c * HPC
        out[lo:lo + HPC] = res.results[c]["out"]
        attn[lo:lo + HPC] = res.results[c]["attn"]

    out = out.reshape(B, H, S, D)
    attn = attn.reshape(B, H, S, S)
    if _trace:
        return (out, attn), res
    return out, attn
